# revision 1
# baseline (speedup 1.0000x reference)
"""Trainium2 Bass kernel for nn_DecoderFactoredLSTM.

Factored-LSTM decoder:
  emb = B_w[captions]                                   [B,T,E] -> tokens [T*B, E]
  u   = emb @ (V^T S^T U^T) + bias                      [T*B, 4H]   (gate pre-activations)
  recurrence over T=40 steps (LSTM, no tanh on c for h)
  out = hiddens @ C_w^T + C_b                           [T*B, V]

Sharding: recurrence + pre-projections replicated on all 8 cores (the
sequential recurrence cannot be sharded without per-step collectives,
whose ~20us latency floor x40 steps dwarfs the compute); the vocab
projection (the dominant FLOPs) is sharded 8-way over vocab columns.
All matmuls run as float32r (reduced-precision fp32 PE mode, ~1.4e-4
rel err, 4x faster than exact fp32).

Column layout for gates everywhere (u, W, gate psum): 8 h-blocks of
512 cols, each block = [i|f|o|ctilde] x 128 h-lanes:
  col(g, h) = (h // 128) * 512 + g * 128 + (h % 128)
"""

import sys

if "/opt/trn_rl_repo" not in sys.path:
    sys.path.insert(0, "/opt/trn_rl_repo")

import numpy as np

import concourse.bass as bass
import concourse.mybir as mybir
import concourse.tile as tile
from concourse import bacc
from concourse.bass import ts, ds
from concourse.bass_utils import run_bass_kernel_spmd
from concourse.masks import make_identity

B, T, E, H, F, V = 64, 40, 512, 1024, 512, 32000
NCORES = 8
VS = V // NCORES  # vocab slice per core: 4000
TOK = T * B  # 2560 tokens
MT = TOK // 128  # 20 token tiles
F32 = mybir.dt.float32
F32R = mybir.dt.float32r

PRO_STEPS = 4  # python-unrolled recurrence prologue steps (incl. t=0)


def _r(ap):
    return ap.bitcast(F32R)


def _build():
    nc = bacc.Bacc(None, target_bir_lowering=False, debug=False)

    with tile.TileContext(nc) as tc:
        cap_d = nc.declare_dram_parameter("cap", [TOK, 1], mybir.dt.int32, isOutput=False)
        Bw_d = nc.declare_dram_parameter("Bw", [V, E], F32, isOutput=False)
        Vg_d = nc.declare_dram_parameter("Vg", [4, F, E], F32R, isOutput=False)
        SgT_d = nc.declare_dram_parameter("SgT", [4, F, F], F32R, isOutput=False)
        UgT_d = nc.declare_dram_parameter("UgT", [4, F, H], F32R, isOutput=False)
        W_d = nc.declare_dram_parameter("Wmov", [H, 4 * H], F32R, isOutput=False)
        ub_d = nc.declare_dram_parameter("ubias", [128, 4 * H], F32, isOutput=False)
        CT_d = nc.declare_dram_parameter("CT", [H, VS], F32R, isOutput=False)
        Cb_d = nc.declare_dram_parameter("Cb", [128, VS], F32, isOutput=False)

        out_d = nc.declare_dram_parameter("out", [TOK, VS], F32, isOutput=True)

        embT_d = nc.dram_tensor("embT", [4, 128, TOK], F32R)  # [e_outer, e_inner, tok]
        u_d = nc.dram_tensor("u", [TOK, 4 * H], F32)
        hT_d = nc.dram_tensor("hT", [8, 128, TOK], F32R)  # [k_outer, k_inner, tok]

        with (
            tc.tile_pool(name="const", bufs=1) as const,
            tc.tile_pool(name="pers", bufs=1) as pers,
        ):
            id128 = const.tile([128, 128], F32, tag="id128")
            make_identity(nc, id128)
            id64 = const.tile([64, 64], F32, tag="id64")
            make_identity(nc, id64)
            # W half A (k-tiles 0..3) prefetched at kernel start, overlapping
            # the gather/M/u phases; half B loads when phase-2 SBUF frees.
            wpoolA = tc.tile_pool(name="wpoolA", bufs=1)
            wpA = wpoolA.__enter__()
            wsbA = wpA.tile([128, 4, 4 * H], F32R, tag="wsbA")
            nc.sync.dma_start(
                wsbA[:], W_d[0 : H // 2, :].rearrange("(ko ki) n -> ki ko n", ki=128)
            )

            # ================= phase 1: gather + transpose emb =================
            with (
                tc.tile_pool(name="ph1", bufs=1) as ph1,
                tc.tile_pool(name="ph1ps", bufs=2, space="PSUM") as ph1ps,
            ):
                idx_all = ph1.tile([128, MT], mybir.dt.int32, tag="idx")
                nc.sync.dma_start(
                    idx_all[:], cap_d[:].rearrange("(m p) o -> p (m o)", p=128)
                )
                for m in range(MT):
                    g_t = ph1.tile([128, E], F32, tag=f"g{m % 3}")
                    nc.gpsimd.indirect_dma_start(
                        out=g_t[:],
                        out_offset=None,
                        in_=Bw_d[:],
                        in_offset=bass.IndirectOffsetOnAxis(
                            ap=idx_all[:, m : m + 1], axis=0
                        ),
                    )
                    stg = ph1.tile([128, 4, 128], F32R, tag=f"stg{m % 3}", name=f"stg{m % 3}")
                    for e in range(4):
                        tp = ph1ps.tile([128, 128], F32, tag=f"tp{e % 2}")
                        nc.tensor.transpose(tp[:], g_t[:, ts(e, 128)], id128[:])
                        nc.vector.tensor_copy(stg[:, e, :], tp[:])
                    nc.sync.dma_start(
                        embT_d[:].rearrange("e ki t -> ki e t")[:, :, ts(m, 128)],
                        stg[:],
                    )

            # ================= phase 2: M = V^T S^T U^T, then u = embT^T @ M ==
            with tc.tile_pool(name="ph2", bufs=1) as ph2:
                mcat = ph2.tile([128, 4, 4 * H], F32R, tag="mcat")
                mpre_sb = tc.tile_pool(name="ph2msb", bufs=1)
                ph2m = mpre_sb.__enter__()
                mpre_ps = tc.tile_pool(name="ph2ps", bufs=2, space="PSUM")
                ph2ps = mpre_ps.__enter__()
                for g in range(4):
                    vg = ph2m.tile([128, 4, E], F32R, tag="vg")
                    nc.sync.dma_start(
                        vg[:], Vg_d[g].rearrange("(ko ki) e -> ki ko e", ki=128)
                    )
                    sgT = ph2m.tile([128, 4, F], F32R, tag="sgT")
                    nc.sync.dma_start(
                        sgT[:], SgT_d[g].rearrange("(ko ki) f -> ki ko f", ki=128)
                    )
                    ugT = ph2m.tile([128, 4, H], F32R, tag="ugT")
                    nc.sync.dma_start(
                        ugT[:], UgT_d[g].rearrange("(ko ki) h -> ki ko h", ki=128)
                    )
                    # PT[f', e] = sum_f S[f',f] V[f,e]; lhsT=S^T [f,f'], rhs=V [f,e]
                    pt = ph2m.tile([128, 4, E], F32R, tag="pt")
                    for fp in range(4):
                        ps = ph2ps.tile([128, E], F32, tag="mp")
                        for k in range(4):
                            nc.tensor.matmul(
                                ps[:],
                                lhsT=sgT[:, k, ts(fp, 128)],
                                rhs=vg[:, k, :],
                                start=(k == 0),
                                stop=(k == 3),
                            )
                        nc.vector.tensor_copy(pt[:, fp, :], ps[:])
                    # M[e, h] = sum_f' PT[f',e] U^T[f',h]
                    for e_t in range(4):
                        for nh in range(2):
                            ps2 = ph2ps.tile([128, 512], F32, tag="mp2")
                            for k in range(4):
                                nc.tensor.matmul(
                                    ps2[:],
                                    lhsT=pt[:, k, ts(e_t, 128)],
                                    rhs=ugT[:, k, ts(nh, 512)],
                                    start=(k == 0),
                                    stop=(k == 3),
                                )
                            for b4 in range(4):
                                blk = nh * 4 + b4
                                nc.vector.tensor_copy(
                                    mcat[:, e_t, blk * 512 + g * 128 : blk * 512 + g * 128 + 128],
                                    ps2[:, ts(b4, 128)],
                                )

                mpre_ps.__exit__(None, None, None)
                mpre_sb.__exit__(None, None, None)
                # u-phase: u[tok, col] = sum_e embT[e, tok] M[e, col] + ubias
                u_sb = tc.tile_pool(name="ph2usb", bufs=1)
                ph2u = u_sb.__enter__()
                u_ps = tc.tile_pool(name="ph2psu", bufs=1, space="PSUM")
                ph2psu = u_ps.__enter__()
                ubias = ph2u.tile([128, 4 * H], F32, tag="ubias")
                nc.sync.dma_start(ubias[:], ub_d[:])
                for m in range(MT):
                    lts = []
                    for k in range(4):
                        lt = ph2u.tile(
                            [128, 128], F32R, tag=f"lt{k}_{m % 3}", name=f"lt{k}_{m % 3}"
                        )
                        nc.scalar.dma_start(lt[:], embT_d[k, :, ts(m, 128)])
                        lts.append(lt)
                    pss = []
                    for n in range(8):
                        pss.append(ph2psu.tile([128, 512], F32, tag=f"up{n}", name=f"up{n}"))
                    for k in range(4):
                        for n in range(8):
                            nc.tensor.matmul(
                                pss[n][:],
                                lhsT=lts[k][:],
                                rhs=mcat[:, k, ts(n, 512)],
                                start=(k == 0),
                                stop=(k == 3),
                            )
                    uev = ph2u.tile([128, 4 * H], F32, tag=f"uev{m % 2}", name=f"uev{m % 2}")
                    for n in range(8):
                        nc.vector.tensor_add(
                            uev[:, ts(n, 512)], pss[n][:], ubias[:, ts(n, 512)]
                        )
                    nc.sync.dma_start(u_d[ts(m, 128), :], uev[:])
                u_ps.__exit__(None, None, None)
                u_sb.__exit__(None, None, None)

            # ================= phase 3: recurrence =================
            with (
                tc.tile_pool(name="ph3", bufs=1) as ph3,
                tc.tile_pool(name="ph3ps", bufs=1, space="PSUM") as ph3ps,
                tc.tile_pool(name="ph3pst", bufs=2, space="PSUM") as ph3pst,
            ):
                wsbB = ph3.tile([128, 4, 4 * H], F32R, tag="wsbB")

                def load_wsbB():
                    for wc in range(2):
                        nc.scalar.dma_start(
                            wsbB[:, ts(wc, 2), :],
                            W_d[
                                H // 2 + wc * 256 : H // 2 + (wc + 1) * 256, :
                            ].rearrange("(ko ki) n -> ki ko n", ki=128),
                        )
                c_sb = pers.tile([64, H], F32, tag="c")
                hidT = [
                    pers.tile([128, 8, 64], F32R, tag="hidTa", name="hidTa"),
                    pers.tile([128, 8, 64], F32R, tag="hidTb", name="hidTb"),
                ]

                def step(t_first, u_slice_rows, hT_cols, parity):
                    """One LSTM step. u_slice_rows/hT_cols: functions giving
                    the dynamic slices; parity: read hidT[1-p], write hidT[p]."""
                    u_t = ph3.tile([64, 4 * H], F32, tag=f"ut{parity}")
                    nc.sync.dma_start(u_t[:], u_d[u_slice_rows, :])
                    rd = hidT[1 - parity]
                    wr = hidT[parity]
                    for nb in range(8):
                        if t_first:
                            src = u_t[:, ts(nb, 512)]
                        else:
                            ps = ph3ps.tile([64, 512], F32, tag=f"rp{nb % 4}")
                            for k in range(8):
                                wtile = wsbA if k < 4 else wsbB
                                nc.tensor.matmul(
                                    ps[:],
                                    lhsT=rd[:, k, :],
                                    rhs=wtile[:, k % 4, ts(nb, 512)],
                                    start=(k == 0),
                                    stop=(k == 7),
                                )
                            gs = ph3.tile([64, 512], F32, tag=f"gs{nb % 2}")
                            nc.vector.tensor_add(gs[:], ps[:], u_t[:, ts(nb, 512)])
                            src = gs[:]
                        sio = ph3.tile([64, 384], F32, tag=f"sio{nb % 2}")
                        nc.scalar.activation(
                            sio[:], src[:, 0:384], mybir.ActivationFunctionType.Sigmoid
                        )
                        tt = ph3.tile([64, 128], F32, tag=f"tt{nb % 2}")
                        nc.scalar.activation(
                            tt[:], src[:, 384:512], mybir.ActivationFunctionType.Tanh
                        )
                        it = ph3.tile([64, 128], F32, tag=f"it{nb % 2}")
                        nc.vector.tensor_mul(it[:], sio[:, 0:128], tt[:])
                        if t_first:
                            nc.vector.tensor_copy(c_sb[:, ts(nb, 128)], it[:])
                        else:
                            fc = ph3.tile([64, 128], F32, tag=f"fc{nb % 2}")
                            nc.vector.tensor_mul(
                                fc[:], sio[:, 128:256], c_sb[:, ts(nb, 128)]
                            )
                            nc.vector.tensor_add(c_sb[:, ts(nb, 128)], fc[:], it[:])
                        hb = ph3.tile([64, 128], F32, tag=f"hb{nb % 2}", name=f"hb{nb % 2}")
                        nc.vector.tensor_mul(hb[:], sio[:, 256:384], c_sb[:, ts(nb, 128)])
                        tp = ph3pst.tile([128, 64], F32, tag="tp64")
                        nc.tensor.transpose(tp[:], hb[:], id64[:])
                        nc.vector.tensor_copy(wr[:, nb, :], tp[:])
                    # one DMA out per step: hT_d[ko, ki, tok_cols] <- wr [ki, ko, b]
                    nc.sync.dma_start(
                        hT_d[:].rearrange("ko ki t -> ki ko t")[:, :, hT_cols],
                        wr[:],
                    )

                for t in range(PRO_STEPS):
                    step(t == 0, slice(t * 64, (t + 1) * 64), slice(t * 64, (t + 1) * 64), t % 2)
                    if t == 0:
                        load_wsbB()
                UNROLL = 12
                n_iters = (T - PRO_STEPS) // UNROLL
                with tc.For_i(
                    0, n_iters, 1, hint_engines=(mybir.EngineType.PE,)
                ) as iv:
                    for j in range(UNROLL):
                        tj = PRO_STEPS + j
                        step(
                            False,
                            ds(iv * (UNROLL * 64) + tj * 64, 64),
                            ds(iv * (UNROLL * 64) + tj * 64, 64),
                            tj % 2,
                        )

            wpoolA.__exit__(None, None, None)

            # ================= phase 4: vocab projection =================
            with (
                tc.tile_pool(name="ph4", bufs=1) as ph4,
                tc.tile_pool(name="ph4ps", bufs=1, space="PSUM") as ph4ps,
            ):
                ctA = ph4.tile([128, 8, VS // 2], F32R, tag="ctA")
                ctB = ph4.tile([128, 8, VS // 2], F32R, tag="ctB")
                Q = VS // 4  # 1000 cols per chunk
                for q in range(4):
                    cth_ = ctA if q < 2 else ctB
                    nc.scalar.dma_start(
                        cth_[:, :, ts(q % 2, Q)],
                        CT_d[:, q * Q : (q + 1) * Q].rearrange(
                            "(ko ki) n -> ki ko n", ki=128
                        ),
                    )
                cb = ph4.tile([128, VS], F32, tag="cb")
                nc.sync.dma_start(cb[:], Cb_d[:])
                NP = VS // 8  # 500
                for m in range(MT):
                    lts = []
                    for k in range(8):
                        lt = ph4.tile(
                            [128, 128], F32R, tag=f"plt{k}_{m % 3}", name=f"plt{k}_{m % 3}"
                        )
                        nc.scalar.dma_start(lt[:], hT_d[k, :, ts(m, 128)])
                        lts.append(lt)
                    pss = []
                    for n in range(8):
                        pss.append(ph4ps.tile([128, NP], F32, tag=f"pp{n}", name=f"pp{n}"))
                    for k in range(8):
                        for n in range(8):
                            cth = ctA if n < 4 else ctB
                            nc.tensor.matmul(
                                pss[n][:],
                                lhsT=lts[k][:],
                                rhs=cth[:, k, ts(n % 4, NP)],
                                start=(k == 0),
                                stop=(k == 7),
                            )
                    pev = ph4.tile([128, VS], F32, tag=f"pev{m % 2}")
                    for n in range(8):
                        nc.vector.tensor_add(
                            pev[:, ts(n, NP)], pss[n][:], cb[:, ts(n, NP)]
                        )
                    nc.sync.dma_start(out_d[ts(m, 128), :], pev[:])

    nc.compile()
    return nc


def kernel(**inputs):
    captions = np.asarray(inputs["captions"])
    B_w = np.asarray(inputs["B_w"], dtype=np.float32)
    V_w = np.asarray(inputs["V_w"], dtype=np.float32)
    V_b = np.asarray(inputs["V_b"], dtype=np.float32)
    S_w = np.asarray(inputs["S_w"], dtype=np.float32)
    S_b = np.asarray(inputs["S_b"], dtype=np.float32)
    U_w = np.asarray(inputs["U_w"], dtype=np.float32)
    U_b = np.asarray(inputs["U_b"], dtype=np.float32)
    W_w = np.asarray(inputs["W_w"], dtype=np.float32)
    W_b = np.asarray(inputs["W_b"], dtype=np.float32)
    C_w = np.asarray(inputs["C_w"], dtype=np.float32)
    C_b = np.asarray(inputs["C_b"], dtype=np.float32)

    # --- host-side layout prep (weights only) ---
    cap = np.ascontiguousarray(captions.T.reshape(TOK, 1)).astype(np.int32)
    SgT = np.ascontiguousarray(S_w.transpose(0, 2, 1))
    UgT = np.ascontiguousarray(U_w.transpose(0, 2, 1))
    # Wmov[k, col(g,h)]: [4,H,K] -> [K, 8, 4, 128] -> [K, 4H]
    Wmov = np.ascontiguousarray(
        W_w.transpose(2, 0, 1).reshape(H, 4, 8, 128).transpose(0, 2, 1, 3).reshape(H, 4 * H)
    )
    # gate bias chain, folded: ((V_b @ S^T + S_b) @ U^T + U_b) + W_b
    bs = np.einsum("gf,gof->go", V_b, S_w) + S_b  # [4, F]
    bu = np.einsum("gf,ghf->gh", bs, U_w) + U_b  # [4, H]
    gate_bias = bu + W_b  # [4, H]
    ub_cols = gate_bias.reshape(4, 8, 128).transpose(1, 0, 2).reshape(4 * H)
    ub_rep = np.ascontiguousarray(np.broadcast_to(ub_cols, (128, 4 * H)))
    CT = np.ascontiguousarray(C_w.T)  # [H, V]

    nc = _build()

    in_maps = []
    for c in range(NCORES):
        in_maps.append(
            {
                "cap": cap,
                "Bw": B_w,
                "Vg": V_w,
                "SgT": SgT,
                "UgT": UgT,
                "Wmov": Wmov,
                "ubias": ub_rep,
                "CT": np.ascontiguousarray(CT[:, c * VS : (c + 1) * VS]),
                "Cb": np.ascontiguousarray(
                    np.broadcast_to(C_b[c * VS : (c + 1) * VS], (128, VS))
                ),
            }
        )

    global _last_in_maps
    _last_in_maps = in_maps

    res = run_bass_kernel_spmd(nc, in_maps, list(range(NCORES)))
    out = np.concatenate([res.results[c]["out"] for c in range(NCORES)], axis=1)
    return out.astype(np.float32)


_last_in_maps = None



# revision 6
# speedup vs baseline: 1.2411x; 1.2411x over previous
"""Trainium2 Bass kernel for nn_DecoderFactoredLSTM (v2: fused bf16 pipeline).

Factored-LSTM decoder:
  emb = B_w[captions]                                   [B,T,E] -> tokens [T*B, E]
  u   = emb @ (V^T S^T U^T) + bias                      [T*B, 4H]   (gate pre-activations)
  recurrence over T=40 steps (LSTM, no tanh on c for h)
  out = hiddens @ C_w^T + C_b                           [T*B, V]

Sharding: recurrence + pre-projections replicated on all 8 cores; the
vocab projection (dominant FLOPs) sharded 8-way over vocab columns.

v2 structure: one fused pipeline. Per m-tile (128 tokens = 2 LSTM steps):
  [rec 2m MMs][u(m+1) chunks 0-3][tp 2m][voc(m-1) 4-7]
  [rec 2m+1 MMs][u(m+1) chunks 4-7][tp 2m+1][voc(m) 0-3]
u-tiles and hidden-transposes live in SBUF (no DRAM roundtrip); the
independent u/vocab matmuls fill the recurrence's elementwise tails so
PE never idles. Weights (W, C^T, M) and activations in bf16 (fp32 PSUM
accumulate; c-state fp32); u/M/phase-1 matmuls f32r where inputs are f32.

Column layout (gate-major): col = g*1024 + h, g in {i,f,o,ctilde}.
Chunk c = g*2 + v covers cols [c*512, (c+1)*512) (h-half v of gate g).
hT kept as pairT[buf][ki, ko, par*64+b] (par = step parity in the m-tile).
"""

import sys
from contextlib import ExitStack

if "/opt/trn_rl_repo" not in sys.path:
    sys.path.insert(0, "/opt/trn_rl_repo")

import numpy as np

import concourse.bass as bass
import concourse.mybir as mybir
import concourse.tile as tile
from concourse import bacc
from concourse.bass import ts
from concourse.bass_utils import run_bass_kernel_spmd
from concourse.masks import make_identity

B, T, E, H, F, V = 64, 40, 512, 1024, 512, 32000
NCORES = 8
VS = V // NCORES  # 4000
TOK = T * B  # 2560
MT = TOK // 128  # 20
NV = VS // 8  # 500 vocab cols per chunk
F32 = mybir.dt.float32
F32R = mybir.dt.float32r
BF16 = mybir.dt.bfloat16
SIG = mybir.ActivationFunctionType.Sigmoid
TANH = mybir.ActivationFunctionType.Tanh
COPY = mybir.ActivationFunctionType.Copy

# chunk emission order per step: ctilde first (tanh feeds the c-chain),
# then i, f, o per half
CHUNK_ORDER = [(3, 0), (0, 0), (1, 0), (2, 0), (3, 1), (0, 1), (1, 1), (2, 1)]


def _build():
    nc = bacc.Bacc(None, target_bir_lowering=False, debug=False)

    with tile.TileContext(nc) as tc:
        cap_d = nc.declare_dram_parameter("cap", [TOK, 1], mybir.dt.int32, isOutput=False)
        Bw_d = nc.declare_dram_parameter("Bw", [V, E], F32, isOutput=False)
        Vg_d = nc.declare_dram_parameter("Vg", [4, F, E], F32R, isOutput=False)
        SgT_d = nc.declare_dram_parameter("SgT", [4, F, F], F32R, isOutput=False)
        UgT_d = nc.declare_dram_parameter("UgT", [4, F, H], F32R, isOutput=False)
        W_d = nc.declare_dram_parameter("Wmov", [H, 4 * H], BF16, isOutput=False)
        ub_d = nc.declare_dram_parameter("ubias", [128, 4 * H], BF16, isOutput=False)
        CT_d = nc.declare_dram_parameter("CT", [H, VS], BF16, isOutput=False)
        out_d = nc.declare_dram_parameter("out", [TOK, VS], BF16, isOutput=True)

        embT_d = nc.dram_tensor("embT", [4, 128, TOK], BF16)  # [e_outer, e_inner, tok]

        with ExitStack() as stack:
            pers = stack.enter_context(tc.tile_pool(name="pers", bufs=1))
            idb = pers.tile([128, 128], BF16, tag="idb")
            make_identity(nc, idb)
            wsbA = pers.tile([128, 4, 4 * H], BF16, tag="wsbA")
            nc.sync.dma_start(
                wsbA[:], W_d[0 : H // 2, :].rearrange("(ko ki) n -> ki ko n", ki=128)
            )
            ctA = pers.tile([128, 4, VS], BF16, tag="ctA")
            nc.sync.dma_start(
                ctA[:], CT_d[0 : H // 2, :].rearrange("(ko ki) n -> ki ko n", ki=128)
            )
            ubias = pers.tile([128, 4 * H], BF16, tag="ubias")
            nc.sync.dma_start(ubias[:], ub_d[:])
            mcat = pers.tile([128, 4, 4 * H], BF16, tag="mcat")
            u_sb = [pers.tile([128, 4 * H], BF16, tag=f"u{i}", name=f"u{i}") for i in range(2)]
            pairT = [
                pers.tile([128, 8, 128], BF16, tag=f"pairT{i}", name=f"pairT{i}")
                for i in range(2)
            ]
            c_sb = pers.tile([64, H], F32, tag="c")

            # ================= prologue: M = V^T S^T U^T, and embT =========
            with (
                tc.tile_pool(name="ph2", bufs=1) as ph2,
                tc.tile_pool(name="ph2ps", bufs=2, space="PSUM") as ph2ps,
                tc.tile_pool(name="ph1", bufs=1) as ph1,
                tc.tile_pool(name="ph1b", bufs=3) as ph1b,
                tc.tile_pool(name="ph1ps", bufs=2, space="PSUM") as ph1ps,
            ):
                idx_all = ph1.tile([128, MT], mybir.dt.int32, tag="idx")
                nc.sync.dma_start(
                    idx_all[:], cap_d[:].rearrange("(m p) o -> p (m o)", p=128)
                )

                def m_gate(g):
                    vg = ph2.tile([128, 4, E], F32R, tag="vg")
                    nc.sync.dma_start(
                        vg[:], Vg_d[g].rearrange("(ko ki) e -> ki ko e", ki=128)
                    )
                    sgT = ph2.tile([128, 4, F], F32R, tag="sgT")
                    nc.sync.dma_start(
                        sgT[:], SgT_d[g].rearrange("(ko ki) f -> ki ko f", ki=128)
                    )
                    ugT = ph2.tile([128, 4, H], F32R, tag="ugT")
                    nc.sync.dma_start(
                        ugT[:], UgT_d[g].rearrange("(ko ki) h -> ki ko h", ki=128)
                    )
                    # PT[f', e] = sum_f S[f',f] V[f,e]
                    pt = ph2.tile([128, 4, E], F32R, tag="pt")
                    for fp in range(4):
                        ps = ph2ps.tile([128, E], F32, tag="mp")
                        for k in range(4):
                            nc.tensor.matmul(
                                ps[:],
                                lhsT=sgT[:, k, ts(fp, 128)],
                                rhs=vg[:, k, :],
                                start=(k == 0),
                                stop=(k == 3),
                            )
                        nc.vector.tensor_copy(pt[:, fp, :], ps[:])
                    # M[e, g*1024 + nh*512 + h'] = sum_f' PT[f',e] U^T[f',h]
                    for e_t in range(4):
                        for nh in range(2):
                            ps2 = ph2ps.tile([128, 512], F32, tag="mp")
                            for k in range(4):
                                nc.tensor.matmul(
                                    ps2[:],
                                    lhsT=pt[:, k, ts(e_t, 128)],
                                    rhs=ugT[:, k, ts(nh, 512)],
                                    start=(k == 0),
                                    stop=(k == 3),
                                )
                            nc.vector.tensor_copy(
                                mcat[:, e_t, g * 1024 + nh * 512 : g * 1024 + (nh + 1) * 512],
                                ps2[:],
                            )

                def ph1_m(m):
                    g_t = ph1b.tile([128, E], F32, tag="gt")
                    nc.gpsimd.indirect_dma_start(
                        out=g_t[:],
                        out_offset=None,
                        in_=Bw_d[:],
                        in_offset=bass.IndirectOffsetOnAxis(
                            ap=idx_all[:, m : m + 1], axis=0
                        ),
                    )
                    gb = ph1b.tile([128, E], BF16, tag="gb")
                    nc.vector.tensor_copy(gb[:], g_t[:])
                    stg = ph1b.tile([128, 4, 128], BF16, tag="stg")
                    for e in range(4):
                        tp = ph1ps.tile([128, 128], BF16, tag="tp1")
                        nc.tensor.transpose(tp[:], gb[:, ts(e, 128)], idb[:])
                        nc.vector.tensor_copy(stg[:, e, :], tp[:])
                    nc.sync.dma_start(
                        embT_d[:].rearrange("e ki t -> ki e t")[:, :, ts(m, 128)],
                        stg[:],
                    )

                for g in range(4):
                    m_gate(g)
                    for m in range(5 * g, 5 * g + 5):
                        ph1_m(m)

            # ================= late weights + main pipeline ================
            late = stack.enter_context(tc.tile_pool(name="late", bufs=1))
            wsbB = late.tile([128, 4, 4 * H], BF16, tag="wsbB")
            nc.sync.dma_start(
                wsbB[:], W_d[H // 2 :, :].rearrange("(ko ki) n -> ki ko n", ki=128)
            )
            ctB = late.tile([128, 4, VS], BF16, tag="ctB")
            nc.sync.dma_start(
                ctB[:], CT_d[H // 2 :, :].rearrange("(ko ki) n -> ki ko n", ki=128)
            )

            ltp = stack.enter_context(tc.tile_pool(name="ltp", bufs=2))
            gsp = stack.enter_context(tc.tile_pool(name="gsp", bufs=2))
            sigp = stack.enter_context(tc.tile_pool(name="sigp", bufs=5))
            itp = stack.enter_context(tc.tile_pool(name="itp", bufs=2))
            fcp = stack.enter_context(tc.tile_pool(name="fcp", bufs=2))
            hcp = stack.enter_context(tc.tile_pool(name="hcp", bufs=4))
            pevp = stack.enter_context(tc.tile_pool(name="pevp", bufs=2))
            recp = stack.enter_context(tc.tile_pool(name="recp", bufs=2, space="PSUM"))
            up = stack.enter_context(tc.tile_pool(name="up", bufs=2, space="PSUM"))
            vp = stack.enter_context(tc.tile_pool(name="vp", bufs=2, space="PSUM"))
            tpp = stack.enter_context(tc.tile_pool(name="tpp", bufs=2, space="PSUM"))

            def load_lt(mt):
                lts = []
                for k in range(4):
                    lt = ltp.tile([128, 128], BF16, tag=f"lt{k}")
                    nc.sync.dma_start(lt[:], embT_d[k, :, ts(mt, 128)])
                    lts.append(lt)
                return lts

            def u_chunks(lts, mt, chunks):
                for c in chunks:
                    ps = up.tile([128, 512], F32, tag="up")
                    for k in range(4):
                        nc.tensor.matmul(
                            ps[:],
                            lhsT=lts[k][:],
                            rhs=mcat[:, k, ts(c, 512)],
                            start=(k == 0),
                            stop=(k == 3),
                        )
                    nc.vector.tensor_add(
                        u_sb[mt % 2][:, ts(c, 512)], ps[:], ubias[:, ts(c, 512)]
                    )

            def rec_step(t):
                """One LSTM step: gate matmuls + elementwise (no transposes)."""
                mb = (t // 2) % 2
                upar = t % 2
                # h_{t-1} source
                pmb = ((t - 1) // 2) % 2
                ppar = (t - 1) % 2
                sg = {}
                for g, v in CHUNK_ORDER:
                    c = g * 2 + v
                    if t == 0:
                        src = u_sb[0][0:64, ts(c, 512)]
                    else:
                        ps = recp.tile([64, 512], F32, tag="rp")
                        for k in range(8):
                            wsbX = wsbA if k < 4 else wsbB
                            nc.tensor.matmul(
                                ps[:],
                                lhsT=pairT[pmb][:, k, ppar * 64 : (ppar + 1) * 64],
                                rhs=wsbX[:, k % 4, ts(c, 512)],
                                start=(k == 0),
                                stop=(k == 7),
                            )
                        gs = gsp.tile([64, 512], BF16, tag="gs")
                        nc.vector.tensor_add(
                            gs[:], ps[:], u_sb[mb][upar * 64 : (upar + 1) * 64, ts(c, 512)]
                        )
                        src = gs[:]
                    sgt = sigp.tile([64, 512], BF16, tag="sig")
                    nc.scalar.activation(sgt[:], src, TANH if g == 3 else SIG)
                    sg[(g, v)] = sgt
                    if g == 2:  # chunks for half v complete -> c-chain + h
                        if t == 0:
                            nc.vector.tensor_mul(
                                c_sb[:, ts(v, 512)], sg[(0, v)][:], sg[(3, v)][:]
                            )
                        else:
                            it = itp.tile([64, 512], BF16, tag="it")
                            nc.vector.tensor_mul(it[:], sg[(0, v)][:], sg[(3, v)][:])
                            fc = fcp.tile([64, 512], BF16, tag="fc")
                            nc.vector.tensor_mul(
                                fc[:], sg[(1, v)][:], c_sb[:, ts(v, 512)]
                            )
                            nc.vector.tensor_add(c_sb[:, ts(v, 512)], fc[:], it[:])
                        hcs = []
                        for q in range(2 * v, 2 * v + 2):
                            hc = hcp.tile([128, 128], BF16, tag="hc")
                            hcs.append(hc)
                        for kt in range(4 * v, 4 * v + 4):
                            q, l = kt // 2, kt % 2
                            nc.vector.tensor_mul(
                                hcs[q - 2 * v][l * 64 : (l + 1) * 64, :],
                                sg[(2, v)][:, ts(kt - 4 * v, 128)],
                                c_sb[:, ts(kt, 128)],
                            )
                        if v == 0:
                            step_hcs[0:2] = hcs
                        else:
                            step_hcs[2:4] = hcs

            step_hcs = [None] * 4

            def transposes(t):
                mb = (t // 2) % 2
                par = t % 2
                for q in range(4):
                    tp = tpp.tile([128, 128], BF16, tag="tp")
                    nc.tensor.transpose(tp[:], step_hcs[q][:], idb[:])
                    for l in range(2):
                        nc.vector.tensor_copy(
                            pairT[mb][:, 2 * q + l, par * 64 : (par + 1) * 64],
                            tp[:, ts(l, 64)],
                        )

            def voc_chunks(vm, chunks):
                for n in chunks:
                    ps = vp.tile([128, NV], F32, tag="vpp")
                    for k in range(8):
                        ctX = ctA if k < 4 else ctB
                        nc.tensor.matmul(
                            ps[:],
                            lhsT=pairT[vm % 2][:, k, :],
                            rhs=ctX[:, k % 4, ts(n, NV)],
                            start=(k == 0),
                            stop=(k == 7),
                        )
                    pev = pevp.tile([128, NV], BF16, tag="pev")
                    nc.scalar.activation(pev[:], ps[:], COPY)
                    nc.gpsimd.dma_start(out_d[ts(vm, 128), ts(n, NV)], pev[:])

            # u(0)
            lts = load_lt(0)
            u_chunks(lts, 0, range(8))

            for m in range(MT):
                lts = load_lt(m + 1) if m + 1 < MT else None
                rec_step(2 * m)
                if lts is not None:
                    u_chunks(lts, m + 1, range(4))
                transposes(2 * m)
                if m >= 1:
                    voc_chunks(m - 1, range(4, 8))
                rec_step(2 * m + 1)
                if lts is not None:
                    u_chunks(lts, m + 1, range(4, 8))
                transposes(2 * m + 1)
                voc_chunks(m, range(4))
            voc_chunks(MT - 1, range(4, 8))

    nc.compile()
    return nc


def kernel(**inputs):
    import ml_dtypes

    BF = ml_dtypes.bfloat16

    captions = np.asarray(inputs["captions"])
    B_w = np.asarray(inputs["B_w"], dtype=np.float32)
    V_w = np.asarray(inputs["V_w"], dtype=np.float32)
    V_b = np.asarray(inputs["V_b"], dtype=np.float32)
    S_w = np.asarray(inputs["S_w"], dtype=np.float32)
    S_b = np.asarray(inputs["S_b"], dtype=np.float32)
    U_w = np.asarray(inputs["U_w"], dtype=np.float32)
    U_b = np.asarray(inputs["U_b"], dtype=np.float32)
    W_w = np.asarray(inputs["W_w"], dtype=np.float32)
    W_b = np.asarray(inputs["W_b"], dtype=np.float32)
    C_w = np.asarray(inputs["C_w"], dtype=np.float32)
    C_b = np.asarray(inputs["C_b"], dtype=np.float32)

    # --- host-side layout prep (weights only) ---
    cap = np.ascontiguousarray(captions.T.reshape(TOK, 1)).astype(np.int32)
    SgT = np.ascontiguousarray(S_w.transpose(0, 2, 1))
    UgT = np.ascontiguousarray(U_w.transpose(0, 2, 1))
    # gate-major columns: col = g*1024 + h
    Wmov = np.ascontiguousarray(
        W_w.transpose(2, 0, 1).reshape(H, 4 * H)
    ).astype(BF)
    # gate bias chain, folded: ((V_b @ S^T + S_b) @ U^T + U_b) + W_b
    bs = np.einsum("gf,gof->go", V_b, S_w) + S_b
    bu = np.einsum("gf,ghf->gh", bs, U_w) + U_b
    gate_bias = (bu + W_b).reshape(4 * H)
    ub_rep = np.ascontiguousarray(np.broadcast_to(gate_bias, (128, 4 * H))).astype(BF)
    CT = np.ascontiguousarray(C_w.T)  # [H, V]

    nc = _build()

    in_maps = []
    for c in range(NCORES):
        in_maps.append(
            {
                "cap": cap,
                "Bw": B_w,
                "Vg": V_w,
                "SgT": SgT,
                "UgT": UgT,
                "Wmov": Wmov,
                "ubias": ub_rep,
                "CT": np.ascontiguousarray(CT[:, c * VS : (c + 1) * VS]).astype(BF),
            }
        )

    global _last_in_maps
    _last_in_maps = in_maps

    res = run_bass_kernel_spmd(nc, in_maps, list(range(NCORES)))
    out = np.concatenate(
        [res.results[c]["out"].astype(np.float32) for c in range(NCORES)], axis=1
    )
    out += C_b[None, :]
    return out


_last_in_maps = None


# revision 15
# speedup vs baseline: 1.4920x; 1.2022x over previous
"""Trainium2 Bass kernel for nn_DecoderFactoredLSTM (v3: col-tiled bf16 pipeline).

Factored-LSTM decoder:
  emb = B_w[captions]                                   [B,T,E] -> tokens [T*B, E]
  u   = emb @ (V^T S^T U^T) + bias                      [T*B, 4H]   (gate pre-activations)
  recurrence over T=40 steps (LSTM, no tanh on c for h)
  out = hiddens @ C_w^T + C_b                           [T*B, V]

Sharding: recurrence + pre-projections replicated on all 8 cores; the
vocab projection (dominant FLOPs) sharded 8-way over vocab columns.

v3 = v2 (fused bf16 pipeline, SBUF-resident u/hT, interleaved fillers)
plus PE column-tiling: batch is 64 so gate/u matmuls only fill half the
128-wide stationary array; pairs of matmuls at tile_position (0,0) and
(0,64) run CONCURRENTLY, nearly halving recurrence+u PE time. Outputs
land stacked on psum partitions p = b + 64*v (v = h-half of the gate),
which also makes every elementwise op a full 128-partition [128,512] op.

Column layout (gate-major): col = g*1024 + 512*v + h'. W/M rhs chunk
(g,v) = cols [(2g+v)*512, ...). hT kept as pairT[buf][ki, ko, par*64+b]
(par = step parity in the m-tile); h-tile [b+64v, (q,h'')] transposes
q-slices into k-tiles ko = q + 4v.
"""

import sys
from contextlib import ExitStack

if "/opt/trn_rl_repo" not in sys.path:
    sys.path.insert(0, "/opt/trn_rl_repo")

import numpy as np

import concourse.bass as bass
import concourse.mybir as mybir
import concourse.tile as tile
from concourse import bacc
from concourse.bass import ts
from concourse.bass_utils import run_bass_kernel_spmd
from concourse.masks import make_identity

B, T, E, H, F, V = 64, 40, 512, 1024, 512, 32000
NCORES = 8
VS = V // NCORES  # 4000
TOK = T * B  # 2560
MT = TOK // 128  # 20
NV = VS // 8  # 500
F32 = mybir.dt.float32
F32R = mybir.dt.float32r
BF16 = mybir.dt.bfloat16
SIG = mybir.ActivationFunctionType.Sigmoid
TANH = mybir.ActivationFunctionType.Tanh
COPY = mybir.ActivationFunctionType.Copy

GATE_ORDER = [3, 0, 1, 2]  # ctilde first (tanh feeds the c-chain), then i, f, o


def _build():
    nc = bacc.Bacc(None, target_bir_lowering=False, debug=False)

    with tile.TileContext(nc) as tc:
        cap_d = nc.declare_dram_parameter("cap", [TOK, 1], mybir.dt.int32, isOutput=False)
        Bw_d = nc.declare_dram_parameter("Bw", [V, E], F32, isOutput=False)
        Vg_d = nc.declare_dram_parameter("Vg", [4, F, E], F32R, isOutput=False)
        SgT_d = nc.declare_dram_parameter("SgT", [4, F, F], F32R, isOutput=False)
        UgT_d = nc.declare_dram_parameter("UgT", [4, F, H], F32R, isOutput=False)
        W_d = nc.declare_dram_parameter("Wmov", [H, 4 * H], BF16, isOutput=False)
        ub_d = nc.declare_dram_parameter("ubias2", [128, 2 * H], BF16, isOutput=False)
        CT_d = nc.declare_dram_parameter("CT", [H, VS], BF16, isOutput=False)
        out_d = nc.declare_dram_parameter("out", [TOK, VS], BF16, isOutput=True)

        embT_d = nc.dram_tensor("embT", [4, 128, TOK], BF16)  # [e_outer, e_inner, tok]

        with ExitStack() as stack:
            pers = stack.enter_context(tc.tile_pool(name="pers", bufs=1))
            idb = pers.tile([128, 128], BF16, tag="idb")
            make_identity(nc, idb)
            # token indices first on the sync queue so gathers start early
            idx_all = pers.tile([128, MT], mybir.dt.int32, tag="idx")
            nc.sync.dma_start(
                idx_all[:], cap_d[:].rearrange("(m p) o -> p (m o)", p=128)
            )
            wsbA = pers.tile([128, 4, 4 * H], BF16, tag="wsbA")
            nc.sync.dma_start(
                wsbA[:], W_d[0 : H // 2, :].rearrange("(ko ki) n -> ki ko n", ki=128)
            )
            wsbB = pers.tile([128, 4, 4 * H], BF16, tag="wsbB")
            nc.sync.dma_start(
                wsbB[:], W_d[H // 2 :, :].rearrange("(ko ki) n -> ki ko n", ki=128)
            )
            ubias = pers.tile([128, 2 * H], BF16, tag="ubias")
            nc.sync.dma_start(ubias[:], ub_d[:])
            mcat = pers.tile([128, 4, 4 * H], BF16, tag="mcat")
            # u2_sb[mparity][stepparity]: stacked u' [b+64v, (g, h')]
            u2_sb = [
                [
                    pers.tile([128, 2 * H], BF16, tag=f"u{i}{s}", name=f"u{i}{s}")
                    for s in range(2)
                ]
                for i in range(2)
            ]
            pairT = [
                pers.tile([128, 8, 128], BF16, tag=f"pairT{i}", name=f"pairT{i}")
                for i in range(2)
            ]
            c_sb = pers.tile([128, 512], F32, tag="c")

            # ================= prologue: M = V^T S^T U^T, and embT =========
            with (
                tc.tile_pool(name="ph2a", bufs=2) as ph2a,
                tc.tile_pool(name="ph2b", bufs=1) as ph2b,
                tc.tile_pool(name="ph2ps", bufs=2, space="PSUM") as ph2ps,
                tc.tile_pool(name="ph1b", bufs=3) as ph1b,
                tc.tile_pool(name="ph1ps", bufs=2, space="PSUM") as ph1ps,
            ):

                def m_gate(g):
                    vg = ph2a.tile([128, 4, E], F32R, tag="vg")
                    nc.scalar.dma_start(
                        vg[:], Vg_d[g].rearrange("(ko ki) e -> ki ko e", ki=128)
                    )
                    sgT = ph2a.tile([128, 4, F], F32R, tag="sgT")
                    nc.scalar.dma_start(
                        sgT[:], SgT_d[g].rearrange("(ko ki) f -> ki ko f", ki=128)
                    )
                    ugT = ph2b.tile([128, 4, H], F32R, tag="ugT")
                    nc.scalar.dma_start(
                        ugT[:], UgT_d[g].rearrange("(ko ki) f -> ki ko f", ki=128)
                    )
                    # PT[f', e] = sum_f S[f',f] V[f,e]
                    pt = ph2b.tile([128, 4, E], F32R, tag="pt")
                    for fp in range(4):
                        ps = ph2ps.tile([128, E], F32, tag="mp")
                        for k in range(4):
                            nc.tensor.matmul(
                                ps[:],
                                lhsT=sgT[:, k, ts(fp, 128)],
                                rhs=vg[:, k, :],
                                start=(k == 0),
                                stop=(k == 3),
                            )
                        nc.vector.tensor_copy(pt[:, fp, :], ps[:])
                    # M[e, col], col = g*1024 + nh*512 + h'
                    for e_t in range(4):
                        for nh in range(2):
                            ps2 = ph2ps.tile([128, 512], F32, tag="mp")
                            for k in range(4):
                                nc.tensor.matmul(
                                    ps2[:],
                                    lhsT=pt[:, k, ts(e_t, 128)],
                                    rhs=ugT[:, k, ts(nh, 512)],
                                    start=(k == 0),
                                    stop=(k == 3),
                                )
                            nc.vector.tensor_copy(
                                mcat[:, e_t, g * 1024 + nh * 512 : g * 1024 + (nh + 1) * 512],
                                ps2[:],
                            )

                def ph1_m(m):
                    g_t = ph1b.tile([128, E], F32, tag="gt")
                    nc.gpsimd.indirect_dma_start(
                        out=g_t[:],
                        out_offset=None,
                        in_=Bw_d[:],
                        in_offset=bass.IndirectOffsetOnAxis(
                            ap=idx_all[:, m : m + 1], axis=0
                        ),
                    )
                    gb = ph1b.tile([128, E], BF16, tag="gb")
                    nc.vector.tensor_copy(gb[:], g_t[:])
                    stg = ph1b.tile([128, 4, 128], BF16, tag="stg")
                    for e in range(4):
                        tp = ph1ps.tile([128, 128], BF16, tag="tp1")
                        nc.tensor.transpose(tp[:], gb[:, ts(e, 128)], idb[:])
                        nc.vector.tensor_copy(stg[:, e, :], tp[:])
                    nc.sync.dma_start(
                        embT_d[:].rearrange("e ki t -> ki e t")[:, :, ts(m, 128)],
                        stg[:],
                    )

                for g in range(4):
                    m_gate(g)
                    for m in range(5 * g, 5 * g + 5):
                        ph1_m(m)

            # ================= late weights + main pipeline ================
            late = stack.enter_context(tc.tile_pool(name="late", bufs=1))
            ctA = late.tile([128, 4, VS], BF16, tag="ctA")
            nc.sync.dma_start(
                ctA[:], CT_d[0 : H // 2, :].rearrange("(ko ki) n -> ki ko n", ki=128)
            )
            ctB = late.tile([128, 4, VS], BF16, tag="ctB")
            nc.sync.dma_start(
                ctB[:], CT_d[H // 2 :, :].rearrange("(ko ki) n -> ki ko n", ki=128)
            )

            ltp = stack.enter_context(tc.tile_pool(name="ltp", bufs=2))
            gsp = stack.enter_context(tc.tile_pool(name="gsp", bufs=2))
            sigp = stack.enter_context(tc.tile_pool(name="sigp", bufs=5))
            itp = stack.enter_context(tc.tile_pool(name="itp", bufs=2))
            fcp = stack.enter_context(tc.tile_pool(name="fcp", bufs=2))
            htp = stack.enter_context(tc.tile_pool(name="htp", bufs=2))
            pevp = stack.enter_context(tc.tile_pool(name="pevp", bufs=2))
            recp = stack.enter_context(tc.tile_pool(name="recp", bufs=2, space="PSUM"))
            up = stack.enter_context(tc.tile_pool(name="up", bufs=2, space="PSUM"))
            vp = stack.enter_context(tc.tile_pool(name="vp", bufs=2, space="PSUM"))
            tpp = stack.enter_context(tc.tile_pool(name="tpp", bufs=2, space="PSUM"))

            def load_lt(mt):
                lts = []
                for k in range(4):
                    lt = ltp.tile([128, 128], BF16, tag=f"lt{k}")
                    nc.sync.dma_start(lt[:], embT_d[k, :, ts(mt, 128)])
                    lts.append(lt)
                return lts

            def u_steps(lts, mt, steps):
                """Produce stacked u' for steps (parities) of m-tile mt."""
                for s in steps:
                    for g in range(4):
                        ps = up.tile([128, 512], F32, tag="up")
                        for k in range(4):
                            lhsT = lts[k][:, s * 64 : (s + 1) * 64]
                            nc.tensor.matmul(
                                ps[0:64, :],
                                lhsT=lhsT,
                                rhs=mcat[:, k, ts(2 * g, 512)],
                                start=(k == 0),
                                stop=(k == 3),
                                tile_position=(0, 0),
                            )
                            nc.tensor.matmul(
                                ps[64:128, :],
                                lhsT=lhsT,
                                rhs=mcat[:, k, ts(2 * g + 1, 512)],
                                start=(k == 0),
                                stop=(k == 3),
                                tile_position=(0, 64),
                            )
                        nc.vector.tensor_add(
                            u2_sb[mt % 2][s][:, ts(g, 512)], ps[:], ubias[:, ts(g, 512)]
                        )

            def rec_step(t):
                """One LSTM step: col-tiled gate matmuls + stacked elementwise."""
                mb = (t // 2) % 2
                upar = t % 2
                pmb = ((t - 1) // 2) % 2
                ppar = (t - 1) % 2
                sg = {}
                for g in GATE_ORDER:
                    if t == 0:
                        src = u2_sb[0][0][:, ts(g, 512)]
                    else:
                        ps = recp.tile([128, 512], F32, tag="rp")
                        for k in range(8):
                            wsbX = wsbA if k < 4 else wsbB
                            lhsT = pairT[pmb][:, k, ppar * 64 : (ppar + 1) * 64]
                            nc.tensor.matmul(
                                ps[0:64, :],
                                lhsT=lhsT,
                                rhs=wsbX[:, k % 4, ts(2 * g, 512)],
                                start=(k == 0),
                                stop=(k == 7),
                                tile_position=(0, 0),
                            )
                            nc.tensor.matmul(
                                ps[64:128, :],
                                lhsT=lhsT,
                                rhs=wsbX[:, k % 4, ts(2 * g + 1, 512)],
                                start=(k == 0),
                                stop=(k == 7),
                                tile_position=(0, 64),
                            )
                        gs = gsp.tile([128, 512], BF16, tag="gs")
                        nc.vector.tensor_add(
                            gs[:], ps[:], u2_sb[mb][upar][:, ts(g, 512)]
                        )
                        src = gs[:]
                    sgt = sigp.tile([128, 512], BF16, tag="sig")
                    nc.scalar.activation(sgt[:], src, TANH if g == 3 else SIG)
                    sg[g] = sgt
                # c-chain + h (all stacked [128, 512])
                if t == 0:
                    nc.vector.tensor_mul(c_sb[:], sg[0][:], sg[3][:])
                else:
                    it = itp.tile([128, 512], BF16, tag="it")
                    nc.vector.tensor_mul(it[:], sg[0][:], sg[3][:])
                    fc = fcp.tile([128, 512], BF16, tag="fc")
                    nc.vector.tensor_mul(fc[:], sg[1][:], c_sb[:])
                    nc.vector.tensor_add(c_sb[:], fc[:], it[:])
                ht = htp.tile([128, 512], BF16, tag="ht")
                nc.vector.tensor_mul(ht[:], sg[2][:], c_sb[:])
                return ht

            def transposes(t, ht):
                mb = (t // 2) % 2
                par = t % 2
                tp = tpp.tile([128, 4, 128], BF16, tag="tp")
                for q in range(4):
                    nc.tensor.transpose(tp[:, q, :], ht[:, ts(q, 128)], idb[:])
                # 2 strided copies (on ACT - DVE is chain-congested):
                # pairT[:, q+4l, par*64+b] <- tp[:, q, l*64+b]
                for l in range(2):
                    nc.scalar.activation(
                        pairT[mb][:, 4 * l : 4 * l + 4, par * 64 : (par + 1) * 64],
                        tp[:, :, ts(l, 64)],
                        COPY,
                    )

            def voc_chunks(vm, chunks):
                for n in chunks:
                    ps = vp.tile([128, NV], F32, tag="vpp")
                    for k in range(8):
                        ctX = ctA if k < 4 else ctB
                        nc.tensor.matmul(
                            ps[:],
                            lhsT=pairT[vm % 2][:, k, :],
                            rhs=ctX[:, k % 4, ts(n, NV)],
                            start=(k == 0),
                            stop=(k == 7),
                        )
                    pev = pevp.tile([128, NV], BF16, tag="pev")
                    nc.scalar.activation(pev[:], ps[:], COPY)
                    nc.gpsimd.dma_start(out_d[ts(vm, 128), ts(n, NV)], pev[:])

            # u(0)
            lts = load_lt(0)
            u_steps(lts, 0, [0, 1])

            for m in range(MT):
                lts = load_lt(m + 1) if m + 1 < MT else None
                ht = rec_step(2 * m)
                if lts is not None:
                    u_steps(lts, m + 1, [0])
                transposes(2 * m, ht)
                if m >= 1:
                    voc_chunks(m - 1, range(4, 8))
                ht = rec_step(2 * m + 1)
                if lts is not None:
                    u_steps(lts, m + 1, [1])
                transposes(2 * m + 1, ht)
                voc_chunks(m, range(4))
            voc_chunks(MT - 1, range(4, 8))

    nc.compile()
    return nc


def kernel(**inputs):
    import ml_dtypes

    BF = ml_dtypes.bfloat16

    captions = np.asarray(inputs["captions"])
    B_w = np.asarray(inputs["B_w"], dtype=np.float32)
    V_w = np.asarray(inputs["V_w"], dtype=np.float32)
    V_b = np.asarray(inputs["V_b"], dtype=np.float32)
    S_w = np.asarray(inputs["S_w"], dtype=np.float32)
    S_b = np.asarray(inputs["S_b"], dtype=np.float32)
    U_w = np.asarray(inputs["U_w"], dtype=np.float32)
    U_b = np.asarray(inputs["U_b"], dtype=np.float32)
    W_w = np.asarray(inputs["W_w"], dtype=np.float32)
    W_b = np.asarray(inputs["W_b"], dtype=np.float32)
    C_w = np.asarray(inputs["C_w"], dtype=np.float32)
    C_b = np.asarray(inputs["C_b"], dtype=np.float32)

    # --- host-side layout prep (weights only) ---
    cap = np.ascontiguousarray(captions.T.reshape(TOK, 1)).astype(np.int32)
    SgT = np.ascontiguousarray(S_w.transpose(0, 2, 1))
    UgT = np.ascontiguousarray(U_w.transpose(0, 2, 1))
    # gate-major columns: col = g*1024 + h
    Wmov = np.ascontiguousarray(W_w.transpose(2, 0, 1).reshape(H, 4 * H)).astype(BF)
    # gate bias chain, folded: ((V_b @ S^T + S_b) @ U^T + U_b) + W_b
    bs = np.einsum("gf,gof->go", V_b, S_w) + S_b
    bu = np.einsum("gf,ghf->gh", bs, U_w) + U_b
    gate_bias = (bu + W_b).reshape(4, 2, 512)
    # stacked bias: ub2[b + 64v, g*512 + h'] = gate_bias[g, v, h']
    ub2 = np.empty((128, 2 * H), np.float32)
    ub2[0:64, :] = np.broadcast_to(gate_bias[:, 0, :].reshape(2 * H), (64, 2 * H))
    ub2[64:128, :] = np.broadcast_to(gate_bias[:, 1, :].reshape(2 * H), (64, 2 * H))
    ub2 = np.ascontiguousarray(ub2).astype(BF)
    CT = np.ascontiguousarray(C_w.T)  # [H, V]

    nc = _build()

    in_maps = []
    for c in range(NCORES):
        in_maps.append(
            {
                "cap": cap,
                "Bw": B_w,
                "Vg": V_w,
                "SgT": SgT,
                "UgT": UgT,
                "Wmov": Wmov,
                "ubias2": ub2,
                "CT": np.ascontiguousarray(CT[:, c * VS : (c + 1) * VS]).astype(BF),
            }
        )

    global _last_in_maps
    _last_in_maps = in_maps

    res = run_bass_kernel_spmd(nc, in_maps, list(range(NCORES)))
    out = np.concatenate(
        [res.results[c]["out"].astype(np.float32) for c in range(NCORES)], axis=1
    )
    out += C_b[None, :]
    return out


_last_in_maps = None


# revision 19
# speedup vs baseline: 1.5308x; 1.0260x over previous
"""Trainium2 Bass kernel for nn_DecoderFactoredLSTM (v3: col-tiled bf16 pipeline).

Factored-LSTM decoder:
  emb = B_w[captions]                                   [B,T,E] -> tokens [T*B, E]
  u   = emb @ (V^T S^T U^T) + bias                      [T*B, 4H]   (gate pre-activations)
  recurrence over T=40 steps (LSTM, no tanh on c for h)
  out = hiddens @ C_w^T + C_b                           [T*B, V]

Sharding: recurrence + pre-projections replicated on all 8 cores; the
vocab projection (dominant FLOPs) sharded 8-way over vocab columns.

v3 = v2 (fused bf16 pipeline, SBUF-resident u/hT, interleaved fillers)
plus PE column-tiling: batch is 64 so gate/u matmuls only fill half the
128-wide stationary array; pairs of matmuls at tile_position (0,0) and
(0,64) run CONCURRENTLY, nearly halving recurrence+u PE time. Outputs
land stacked on psum partitions p = b + 64*v (v = h-half of the gate),
which also makes every elementwise op a full 128-partition [128,512] op.

Column layout (gate-major): col = g*1024 + 512*v + h'. W/M rhs chunk
(g,v) = cols [(2g+v)*512, ...). hT kept as pairT[buf][ki, ko, par*64+b]
(par = step parity in the m-tile); h-tile [b+64v, (q,h'')] transposes
q-slices into k-tiles ko = q + 4v.
"""

import sys
from contextlib import ExitStack

if "/opt/trn_rl_repo" not in sys.path:
    sys.path.insert(0, "/opt/trn_rl_repo")

import numpy as np

import concourse.bass as bass
import concourse.mybir as mybir
import concourse.tile as tile
from concourse import bacc
from concourse.bass import ts
from concourse.bass_utils import run_bass_kernel_spmd
from concourse.masks import make_identity

B, T, E, H, F, V = 64, 40, 512, 1024, 512, 32000
NCORES = 8
VS = V // NCORES  # 4000
TOK = T * B  # 2560
MT = TOK // 128  # 20
NV = VS // 8  # 500
F32 = mybir.dt.float32
F32R = mybir.dt.float32r
BF16 = mybir.dt.bfloat16
SIG = mybir.ActivationFunctionType.Sigmoid
TANH = mybir.ActivationFunctionType.Tanh
COPY = mybir.ActivationFunctionType.Copy

GATE_ORDER = [3, 0, 1, 2]  # ctilde first (tanh feeds the c-chain), then i, f, o


def _build():
    nc = bacc.Bacc(None, target_bir_lowering=False, debug=False)

    with tile.TileContext(nc) as tc:
        cap_d = nc.declare_dram_parameter("cap", [TOK, 1], mybir.dt.int32, isOutput=False)
        Bw_d = nc.declare_dram_parameter("Bw", [V, E], F32, isOutput=False)
        Vg_d = nc.declare_dram_parameter("Vg", [4, F, E], F32R, isOutput=False)
        SgT_d = nc.declare_dram_parameter("SgT", [4, F, F], F32R, isOutput=False)
        UgT_d = nc.declare_dram_parameter("UgT", [4, F, H], F32R, isOutput=False)
        W_d = nc.declare_dram_parameter("Wmov", [H, 4 * H], BF16, isOutput=False)
        ub_d = nc.declare_dram_parameter("ubias2", [128, 2 * H], BF16, isOutput=False)
        CT_d = nc.declare_dram_parameter("CT", [H, VS], BF16, isOutput=False)
        out_d = nc.declare_dram_parameter("out", [TOK, VS], BF16, isOutput=True)

        embT_d = nc.dram_tensor("embT", [4, 128, TOK], BF16)  # [e_outer, e_inner, tok]

        with ExitStack() as stack:
            pers = stack.enter_context(tc.tile_pool(name="pers", bufs=1))
            idb = pers.tile([128, 128], BF16, tag="idb")
            make_identity(nc, idb)
            # token indices first on the sync queue so gathers start early
            idx_all = pers.tile([128, MT], mybir.dt.int32, tag="idx")
            nc.sync.dma_start(
                idx_all[:], cap_d[:].rearrange("(m p) o -> p (m o)", p=128)
            )
            wsbA = pers.tile([128, 4, 4 * H], BF16, tag="wsbA")
            nc.sync.dma_start(
                wsbA[:], W_d[0 : H // 2, :].rearrange("(ko ki) n -> ki ko n", ki=128)
            )
            wsbB = pers.tile([128, 4, 4 * H], BF16, tag="wsbB")
            nc.sync.dma_start(
                wsbB[:], W_d[H // 2 :, :].rearrange("(ko ki) n -> ki ko n", ki=128)
            )
            ubias = pers.tile([128, 2 * H], BF16, tag="ubias")
            nc.sync.dma_start(ubias[:], ub_d[:])
            mcat = pers.tile([128, 4, 4 * H], BF16, tag="mcat")
            # u2_sb[mparity][stepparity]: stacked u' [b+64v, (g, h')]
            u2_sb = [
                [
                    pers.tile([128, 2 * H], BF16, tag=f"u{i}{s}", name=f"u{i}{s}")
                    for s in range(2)
                ]
                for i in range(2)
            ]
            pairT = [
                pers.tile([128, 8, 128], BF16, tag=f"pairT{i}", name=f"pairT{i}")
                for i in range(2)
            ]
            c_sb = pers.tile([128, 512], F32, tag="c")

            # ================= prologue: M = V^T S^T U^T, and embT =========
            with (
                tc.tile_pool(name="ph2a", bufs=2) as ph2a,
                tc.tile_pool(name="ph2u", bufs=2) as ph2u,
                tc.tile_pool(name="ph2b", bufs=1) as ph2b,
                tc.tile_pool(name="ph2ps", bufs=2, space="PSUM") as ph2ps,
                tc.tile_pool(name="ph1b", bufs=2) as ph1b,
                tc.tile_pool(name="ph1ps", bufs=2, space="PSUM") as ph1ps,
            ):

                def m_gate(g):
                    vg = ph2a.tile([128, 4, E], F32R, tag="vg")
                    nc.scalar.dma_start(
                        vg[:], Vg_d[g].rearrange("(ko ki) e -> ki ko e", ki=128)
                    )
                    sgT = ph2a.tile([128, 4, F], F32R, tag="sgT")
                    nc.scalar.dma_start(
                        sgT[:], SgT_d[g].rearrange("(ko ki) f -> ki ko f", ki=128)
                    )
                    ugT = ph2u.tile([128, 4, H], F32R, tag="ugT")
                    nc.scalar.dma_start(
                        ugT[:], UgT_d[g].rearrange("(ko ki) f -> ki ko f", ki=128)
                    )
                    # PT[f', e] = sum_f S[f',f] V[f,e]
                    pt = ph2b.tile([128, 4, E], F32R, tag="pt")
                    for fp in range(4):
                        ps = ph2ps.tile([128, E], F32, tag="mp")
                        for k in range(4):
                            nc.tensor.matmul(
                                ps[:],
                                lhsT=sgT[:, k, ts(fp, 128)],
                                rhs=vg[:, k, :],
                                start=(k == 0),
                                stop=(k == 3),
                            )
                        nc.vector.tensor_copy(pt[:, fp, :], ps[:])
                    # M[e, col], col = g*1024 + nh*512 + h'
                    for e_t in range(4):
                        for nh in range(2):
                            ps2 = ph2ps.tile([128, 512], F32, tag="mp")
                            for k in range(4):
                                nc.tensor.matmul(
                                    ps2[:],
                                    lhsT=pt[:, k, ts(e_t, 128)],
                                    rhs=ugT[:, k, ts(nh, 512)],
                                    start=(k == 0),
                                    stop=(k == 3),
                                )
                            nc.vector.tensor_copy(
                                mcat[:, e_t, g * 1024 + nh * 512 : g * 1024 + (nh + 1) * 512],
                                ps2[:],
                            )

                def ph1_m(m):
                    g_t = ph1b.tile([128, E], F32, tag="gt")
                    nc.gpsimd.indirect_dma_start(
                        out=g_t[:],
                        out_offset=None,
                        in_=Bw_d[:],
                        in_offset=bass.IndirectOffsetOnAxis(
                            ap=idx_all[:, m : m + 1], axis=0
                        ),
                    )
                    gb = ph1b.tile([128, E], BF16, tag="gb")
                    nc.vector.tensor_copy(gb[:], g_t[:])
                    stg = ph1b.tile([128, 4, 128], BF16, tag="stg")
                    for e in range(4):
                        tp = ph1ps.tile([128, 128], BF16, tag="tp1")
                        nc.tensor.transpose(tp[:], gb[:, ts(e, 128)], idb[:])
                        nc.vector.tensor_copy(stg[:, e, :], tp[:])
                    nc.sync.dma_start(
                        embT_d[:].rearrange("e ki t -> ki e t")[:, :, ts(m, 128)],
                        stg[:],
                    )

                for g in range(4):
                    m_gate(g)
                    for m in range(5 * g, 5 * g + 5):
                        ph1_m(m)

            # ================= main pipeline pools =========================
            ltp = stack.enter_context(tc.tile_pool(name="ltp", bufs=2))
            gsp = stack.enter_context(tc.tile_pool(name="gsp", bufs=2))
            sigp = stack.enter_context(tc.tile_pool(name="sigp", bufs=5))
            itp = stack.enter_context(tc.tile_pool(name="itp", bufs=2))
            fcp = stack.enter_context(tc.tile_pool(name="fcp", bufs=2))
            htp = stack.enter_context(tc.tile_pool(name="htp", bufs=2))
            pevp = stack.enter_context(tc.tile_pool(name="pevp", bufs=2))
            recp = stack.enter_context(tc.tile_pool(name="recp", bufs=3, space="PSUM"))
            up = stack.enter_context(tc.tile_pool(name="up", bufs=2, space="PSUM"))
            vp = stack.enter_context(tc.tile_pool(name="vp", bufs=2, space="PSUM"))
            tpp = stack.enter_context(tc.tile_pool(name="tpp", bufs=1, space="PSUM"))

            def load_lt(mt):
                lts = []
                for k in range(4):
                    lt = ltp.tile([128, 128], BF16, tag=f"lt{k}")
                    nc.scalar.dma_start(lt[:], embT_d[k, :, ts(mt, 128)])
                    lts.append(lt)
                return lts

            def u_steps(lts, mt, steps):
                """Produce stacked u' for steps (parities) of m-tile mt."""
                for s in steps:
                    for g in range(4):
                        ps = up.tile([128, 512], F32, tag="up")
                        for k in range(4):
                            lhsT = lts[k][:, s * 64 : (s + 1) * 64]
                            nc.tensor.matmul(
                                ps[0:64, :],
                                lhsT=lhsT,
                                rhs=mcat[:, k, ts(2 * g, 512)],
                                start=(k == 0),
                                stop=(k == 3),
                                tile_position=(0, 0),
                            )
                            nc.tensor.matmul(
                                ps[64:128, :],
                                lhsT=lhsT,
                                rhs=mcat[:, k, ts(2 * g + 1, 512)],
                                start=(k == 0),
                                stop=(k == 3),
                                tile_position=(0, 64),
                            )
                        nc.vector.tensor_add(
                            u2_sb[mt % 2][s][:, ts(g, 512)], ps[:], ubias[:, ts(g, 512)]
                        )

            def rec_step(t):
                """One LSTM step: col-tiled gate matmuls + stacked elementwise."""
                mb = (t // 2) % 2
                upar = t % 2
                pmb = ((t - 1) // 2) % 2
                ppar = (t - 1) % 2
                sg = {}
                for g in GATE_ORDER:
                    if t == 0:
                        src = u2_sb[0][0][:, ts(g, 512)]
                    else:
                        ps = recp.tile([128, 512], F32, tag="rp")
                        for k in range(8):
                            wsbX = wsbA if k < 4 else wsbB
                            lhsT = pairT[pmb][:, k, ppar * 64 : (ppar + 1) * 64]
                            nc.tensor.matmul(
                                ps[0:64, :],
                                lhsT=lhsT,
                                rhs=wsbX[:, k % 4, ts(2 * g, 512)],
                                start=(k == 0),
                                stop=(k == 7),
                                tile_position=(0, 0),
                            )
                            nc.tensor.matmul(
                                ps[64:128, :],
                                lhsT=lhsT,
                                rhs=wsbX[:, k % 4, ts(2 * g + 1, 512)],
                                start=(k == 0),
                                stop=(k == 7),
                                tile_position=(0, 64),
                            )
                        gs = gsp.tile([128, 512], BF16, tag="gs")
                        nc.vector.tensor_add(
                            gs[:], ps[:], u2_sb[mb][upar][:, ts(g, 512)]
                        )
                        src = gs[:]
                    sgt = sigp.tile([128, 512], BF16, tag="sig")
                    nc.scalar.activation(sgt[:], src, TANH if g == 3 else SIG)
                    sg[g] = sgt
                # c-chain + h (all stacked [128, 512])
                if t == 0:
                    nc.vector.tensor_mul(c_sb[:], sg[0][:], sg[3][:])
                else:
                    it = itp.tile([128, 512], BF16, tag="it")
                    nc.vector.tensor_mul(it[:], sg[0][:], sg[3][:])
                    fc = fcp.tile([128, 512], BF16, tag="fc")
                    nc.vector.tensor_mul(fc[:], sg[1][:], c_sb[:])
                    nc.vector.tensor_add(c_sb[:], fc[:], it[:])
                ht = htp.tile([128, 512], BF16, tag="ht")
                nc.vector.tensor_mul(ht[:], sg[2][:], c_sb[:])
                return ht

            def transposes(t, ht):
                mb = (t // 2) % 2
                par = t % 2
                tp = tpp.tile([128, 4, 128], BF16, tag="tp")
                for q in range(4):
                    nc.tensor.transpose(tp[:, q, :], ht[:, ts(q, 128)], idb[:])
                # 2 strided copies (on ACT - DVE is chain-congested):
                # pairT[:, q+4l, par*64+b] <- tp[:, q, l*64+b]
                for l in range(2):
                    nc.scalar.activation(
                        pairT[mb][:, 4 * l : 4 * l + 4, par * 64 : (par + 1) * 64],
                        tp[:, :, ts(l, 64)],
                        COPY,
                    )

            def voc_chunks(vm, chunks):
                for n in chunks:
                    ps = vp.tile([128, NV], F32, tag="vpp")
                    for k in range(8):
                        ctX = ctA if k < 4 else ctB
                        nc.tensor.matmul(
                            ps[:],
                            lhsT=pairT[vm % 2][:, k, :],
                            rhs=ctX[:, k % 4, ts(n, NV)],
                            start=(k == 0),
                            stop=(k == 7),
                        )
                    pev = pevp.tile([128, NV], BF16, tag="pev")
                    nc.scalar.activation(pev[:], ps[:], COPY)
                    nc.gpsimd.dma_start(out_d[ts(vm, 128), ts(n, NV)], pev[:])

            # u(0) before the big late-weight DMAs so it isn't queued
            # behind 8MB of C^T transfer
            lts = load_lt(0)
            u_steps(lts, 0, [0, 1])

            # late vocab weights (transfers overlap the first iterations)
            late = stack.enter_context(tc.tile_pool(name="late", bufs=1))
            ctA = late.tile([128, 4, VS], BF16, tag="ctA")
            nc.sync.dma_start(
                ctA[:], CT_d[0 : H // 2, :].rearrange("(ko ki) n -> ki ko n", ki=128)
            )
            ctB = late.tile([128, 4, VS], BF16, tag="ctB")
            nc.sync.dma_start(
                ctB[:], CT_d[H // 2 :, :].rearrange("(ko ki) n -> ki ko n", ki=128)
            )

            for m in range(MT):
                lts = load_lt(m + 1) if m + 1 < MT else None
                ht = rec_step(2 * m)
                if lts is not None:
                    u_steps(lts, m + 1, [0])
                transposes(2 * m, ht)
                if m >= 1:
                    voc_chunks(m - 1, range(4, 8))
                ht = rec_step(2 * m + 1)
                if lts is not None:
                    u_steps(lts, m + 1, [1])
                transposes(2 * m + 1, ht)
                voc_chunks(m, range(4))
            voc_chunks(MT - 1, range(4, 8))

    nc.compile()
    return nc


def kernel(**inputs):
    import ml_dtypes

    BF = ml_dtypes.bfloat16

    captions = np.asarray(inputs["captions"])
    B_w = np.asarray(inputs["B_w"], dtype=np.float32)
    V_w = np.asarray(inputs["V_w"], dtype=np.float32)
    V_b = np.asarray(inputs["V_b"], dtype=np.float32)
    S_w = np.asarray(inputs["S_w"], dtype=np.float32)
    S_b = np.asarray(inputs["S_b"], dtype=np.float32)
    U_w = np.asarray(inputs["U_w"], dtype=np.float32)
    U_b = np.asarray(inputs["U_b"], dtype=np.float32)
    W_w = np.asarray(inputs["W_w"], dtype=np.float32)
    W_b = np.asarray(inputs["W_b"], dtype=np.float32)
    C_w = np.asarray(inputs["C_w"], dtype=np.float32)
    C_b = np.asarray(inputs["C_b"], dtype=np.float32)

    # --- host-side layout prep (weights only) ---
    cap = np.ascontiguousarray(captions.T.reshape(TOK, 1)).astype(np.int32)
    SgT = np.ascontiguousarray(S_w.transpose(0, 2, 1))
    UgT = np.ascontiguousarray(U_w.transpose(0, 2, 1))
    # gate-major columns: col = g*1024 + h
    Wmov = np.ascontiguousarray(W_w.transpose(2, 0, 1).reshape(H, 4 * H)).astype(BF)
    # gate bias chain, folded: ((V_b @ S^T + S_b) @ U^T + U_b) + W_b
    bs = np.einsum("gf,gof->go", V_b, S_w) + S_b
    bu = np.einsum("gf,ghf->gh", bs, U_w) + U_b
    gate_bias = (bu + W_b).reshape(4, 2, 512)
    # stacked bias: ub2[b + 64v, g*512 + h'] = gate_bias[g, v, h']
    ub2 = np.empty((128, 2 * H), np.float32)
    ub2[0:64, :] = np.broadcast_to(gate_bias[:, 0, :].reshape(2 * H), (64, 2 * H))
    ub2[64:128, :] = np.broadcast_to(gate_bias[:, 1, :].reshape(2 * H), (64, 2 * H))
    ub2 = np.ascontiguousarray(ub2).astype(BF)
    CT = np.ascontiguousarray(C_w.T)  # [H, V]

    nc = _build()

    in_maps = []
    for c in range(NCORES):
        in_maps.append(
            {
                "cap": cap,
                "Bw": B_w,
                "Vg": V_w,
                "SgT": SgT,
                "UgT": UgT,
                "Wmov": Wmov,
                "ubias2": ub2,
                "CT": np.ascontiguousarray(CT[:, c * VS : (c + 1) * VS]).astype(BF),
            }
        )

    global _last_in_maps
    _last_in_maps = in_maps

    res = run_bass_kernel_spmd(nc, in_maps, list(range(NCORES)))
    out = np.concatenate(
        [res.results[c]["out"].astype(np.float32) for c in range(NCORES)], axis=1
    )
    out += C_b[None, :]
    return out


_last_in_maps = None
